# revision 32
# baseline (speedup 1.0000x reference)
"""Inner-policy-sharded Trainium2 kernel for DecoupledDynamicsModel (MoE).

Model: B=8192 rows; each row selects one of P=8 outer policies via
policy_indices; the selected policy runs 8 inner MLPs (72 -> 512 -> 512 -> 64)
on (latent chunk, action) and the 8 inner outputs concatenate to 512 dims.

Sharding: by INNER policy. Core i computes inner MLP i for every row, using
the row's outer-policy weight set W*[outer, i]. Rows are sorted by outer
policy on the host so tokens form 8 contiguous groups; within a group the
weights are stationary. This gives perfect load balance (every core runs
exactly B tokens) and no capacity padding, unlike outer-expert sharding
which pads every core to max(group size).

On-chip layout is feature-major ([features(part), tokens(free)]) so no
transposes are needed between layers. Matmuls run in float32r (FP32 stored,
FP22 multiply, FP32 accumulate) at 1 cycle/row for moving dim >=256.
Bias+relu rides the PSUM->SBUF eviction, split across ACT (layer 1) and
DVE (layers 2 and 3) to balance engine load.

All 8 outer weight sets for this core's inner MLP are packed into eight
[128, 2825]-column SBUF tiles (W1|W2|W3|b1|b2|b3) loaded once up front;
x rides a separate [72, B] tile. Built on Bacc so multi-wait instructions
are legalized (TRN2 allows one sync wait per instruction).
"""

import sys

sys.path.insert(0, "/opt/trn_rl_repo")

import numpy as np

import concourse.bass as bass
from concourse import bacc
import concourse.mybir as mybir
import concourse.tile as tile
from concourse.bass import ts
from concourse.bass_utils import run_bass_kernel_spmd

P = 8          # outer policies == n_cores == inner MLPs per policy
Z = 64         # per-policy latent dim
D = P * Z      # 512
A = 8          # action dim
IN = Z + A     # 72, MLP input dim
H = 512        # hidden dim
NCORES = 8

# column layout of the [128, WBC] packed per-outer weight tile; ordered so
# a tiny leading DMA chunk (biases + W1 m-chunk 0) unblocks the first
# matmul+relu as early as possible
B1_OFF = 0          # [128, 4] (m-chunk per column)
B2_OFF = 4          # [128, 4]
B3_OFF = 8          # [64, 1] (rows 64.. zero)
W1_OFF = 16         # [72, 512] (rows 72.. zero), cols 16..528
W2_OFF = 528        # 4 k-chunks of [128, 512]
W3_OFF = 2576       # 4 k-chunks of [128, 64]
WBC = 2832

F32 = mybir.dt.float32
F32R = mybir.dt.float32r
RELU = mybir.ActivationFunctionType.Relu

TRACE = False
REPEAT = 1
LAST_RESULT = None


def _group_tiles(counts):
    """Token tiles for the sorted stream: each tile stays inside one outer-
    policy group; tiles <=512 and >=256 where possible (float32r needs a
    moving dim >=256 for full rate)."""
    tiles = []
    off = 0
    for g, n in enumerate(counts):
        r = n
        if g == 0 and r > 768:
            # small leading tile: compute starts on fewer loaded bytes and
            # the first relu chain is shorter, so the pipeline ramps sooner
            tiles.append((g, off, 256))
            off += 256
            r -= 256
        while r > 0:
            if r > 768:
                t = 512
            elif r > 512:
                t = r - 256
            else:
                t = r
            tiles.append((g, off, t))
            off += t
            r -= t
    return tiles


def _build_program(counts, B, repeat=1):
    tiles = _group_tiles(counts)
    nc = bacc.Bacc()

    xTd = nc.declare_dram_parameter("xT", [IN, B], F32R, isOutput=False)
    wbd = nc.declare_dram_parameter("wb", [P, 128, WBC], F32R, isOutput=False)
    yTd = nc.declare_dram_parameter("yT", [Z, B], F32, isOutput=True)

    with tile.TileContext(nc) as tc:
        with (
            tc.tile_pool(name="weights", bufs=8) as wpool,
            tc.tile_pool(name="xs", bufs=1) as xpool,
            tc.tile_pool(name="hs", bufs=4) as hpool,
            tc.tile_pool(name="ys", bufs=8) as ypool,
            tc.tile_pool(name="ps1", bufs=4, space="PSUM") as pspool1,
            tc.tile_pool(name="ps2", bufs=3, space="PSUM") as pspool2,
            tc.tile_pool(name="ps3", bufs=1, space="PSUM") as pspool3,
        ):
            for _rep in range(repeat):
                xt = xpool.tile([IN, B], F32R, tag="x")
                wbs = []
                for _g in range(P):
                    wb_t = wpool.tile([128, WBC], F32R, tag="wb")
                    wbs.append(wb_t)
                # DMAs effectively drain through one serial pipe, so emit in
                # NEED order: tile 0's x columns and W1 first, W2 k-chunks
                # interleaved with the next x spans, bulk weights last.
                wb0_cuts = [0, W1_OFF + 256, W2_OFF, W2_OFF + 512,
                            W2_OFF + 1024, W2_OFF + 1536, W3_OFF, WBC]
                x_cuts = [0, 256, 1024, 2048, 4096, B]
                x_cuts = sorted(set(min(c, B) for c in x_cuts))
                xsp = [c for c in zip(x_cuts[:-1], x_cuts[1:]) if c[1] > c[0]]
                w0 = list(zip(wb0_cuts[:-1], wb0_cuts[1:]))
                # serial DMA pipe: emit in need order. Group 0's chunks and
                # the first x spans lead; each later group's chunks follow in
                # group order (just-in-time), remaining x spans interleaved.
                # the first x span rides SWDGE (gpsimd is otherwise idle):
                # it transfers in parallel with the HWDGE weight pipe, so the
                # first matmul is gated only by the leading weight chunk
                nc.gpsimd.dma_start(
                    xt[:, xsp[0][0] : xsp[0][1]], xTd[:, xsp[0][0] : xsp[0][1]]
                )
                # second W1 chunk also on SWDGE: needed ~4us in, so its
                # higher first-byte latency is hidden, and it leaves the
                # HWDGE pipe free for the W2 chunks
                nc.gpsimd.dma_start(wbs[0][:, w0[1][0] : w0[1][1]],
                                    wbd[0, :, w0[1][0] : w0[1][1]])
                order = [(0, w0[0]), (0, w0[2])]
                if len(xsp) > 1:
                    order.append(("x", xsp[1]))
                order += [(0, c) for c in w0[3:]]
                gchunks = [(0, 528), (528, 1552), (1552, 2576), (2576, WBC)]
                for g in range(1, P):
                    order += [(g, c) for c in gchunks]
                    xi = 1 + g
                    if xi < len(xsp):
                        order.append(("x", xsp[xi]))
                for kind, (c0, c1) in order:
                    if c1 <= c0:
                        continue
                    if kind == "x":
                        nc.sync.dma_start(xt[:, c0:c1], xTd[:, c0:c1])
                    else:
                        nc.sync.dma_start(wbs[kind][:, c0:c1], wbd[kind, :, c0:c1])

                for (g, t0, tw) in tiles:
                    wb = wbs[g]
                    h1 = hpool.tile([128, 4, 512], F32R, tag="h1")
                    h2 = hpool.tile([128, 4, 512], F32R, tag="h2")
                    # layer 1: h1 = relu(W1.T @ x + b1), K=72
                    for m in range(4):
                        ps = pspool1.tile([128, 512], F32, tag="ps1")
                        nc.tensor.matmul(
                            ps[:, :tw],
                            wb[0:IN, W1_OFF + m * 128 : W1_OFF + (m + 1) * 128],
                            xt[:, t0 : t0 + tw],
                            start=True,
                            stop=True,
                        )
                        nc.scalar.activation(
                            h1[:, m, :tw], ps[:, :tw], RELU,
                            bias=wb[:, B1_OFF + m : B1_OFF + m + 1].bitcast(F32),
                        )
                    # layer 2: h2 = relu(W2.T @ h1 + b2), K=512 over 4 chunks
                    for m in range(4):
                        ps = pspool2.tile([128, 512], F32, tag="ps2")
                        for k in range(4):
                            nc.tensor.matmul(
                                ps[:, :tw],
                                wb[:, W2_OFF + k * 512 + m * 128 : W2_OFF + k * 512 + (m + 1) * 128],
                                h1[:, k, :tw],
                                start=(k == 0),
                                stop=(k == 3),
                            )
                        nc.vector.tensor_scalar(
                            h2[:, m, :tw],
                            ps[:, :tw],
                            wb[:, B2_OFF + m : B2_OFF + m + 1].bitcast(F32),
                            0.0,
                            mybir.AluOpType.add,
                            mybir.AluOpType.max,
                        )
                    # layer 3: y = W3.T @ h2 + b3, M=64
                    ps = pspool3.tile([Z, 512], F32, tag="ps3")
                    for k in range(4):
                        nc.tensor.matmul(
                            ps[:, :tw],
                            wb[:, W3_OFF + k * Z : W3_OFF + (k + 1) * Z],
                            h2[:, k, :tw],
                            start=(k == 0),
                            stop=(k == 3),
                        )
                    y = ypool.tile([Z, 512], F32, tag="y")
                    nc.vector.tensor_scalar(
                        y[:, :tw], ps[:, :tw],
                        wb[0:Z, B3_OFF : B3_OFF + 1].bitcast(F32),
                        None,
                        mybir.AluOpType.add,
                    )
                    nc.sync.dma_start(yTd[:, t0 : t0 + tw], y[:, :tw])

    nc.finalize()
    return nc


def _pack_inputs(latents, actions, order, counts, pcounts, Bp,
                 W1, b1, W2, b2, W3, b3):
    """Per-core inputs. Core i: xT = [latent chunk i; action] for all rows in
    sorted order (groups padded to pcounts); wb[g] = weights of (outer g,
    inner i)."""
    lat_s = latents[order]                       # [B, 512]
    act_s = actions[order]                       # [B, 8]
    spans = []                                   # (padded off, raw off, n)
    po = ro = 0
    for n, pn in zip(counts, pcounts):
        spans.append((po, ro, n))
        po += pn
        ro += n
    in_maps = []
    for i in range(NCORES):
        xT = np.zeros((IN, Bp), dtype=np.float32)
        for po, ro, n in spans:
            xT[:Z, po : po + n] = lat_s[ro : ro + n, i * Z : (i + 1) * Z].T
            xT[Z:, po : po + n] = act_s[ro : ro + n].T

        wb = np.zeros((P, 128, WBC), dtype=np.float32)
        wb[:, :IN, W1_OFF : W1_OFF + 512] = W1[:, i]           # [P, 72, 512]
        wb[:, :, W2_OFF : W2_OFF + 2048] = (
            W2[:, i].reshape(P, 4, 128, H).transpose(0, 2, 1, 3).reshape(P, 128, 2048)
        )
        wb[:, :, W3_OFF : W3_OFF + 256] = (
            W3[:, i].reshape(P, 4, 128, Z).transpose(0, 2, 1, 3).reshape(P, 128, 256)
        )
        wb[:, :, B1_OFF : B1_OFF + 4] = b1[:, i].reshape(P, 4, 128).transpose(0, 2, 1)
        wb[:, :, B2_OFF : B2_OFF + 4] = b2[:, i].reshape(P, 4, 128).transpose(0, 2, 1)
        wb[:, :Z, B3_OFF] = b3[:, i]

        in_maps.append({"xT": xT, "wb": wb})
    return in_maps


def _prepare(latents, actions, policy_indices, W1, b1, W2, b2, W3, b3):
    latents = np.asarray(latents, dtype=np.float32)
    actions = np.asarray(actions, dtype=np.float32)
    idx = np.asarray(policy_indices).astype(np.int64)
    W1 = np.ascontiguousarray(np.asarray(W1, dtype=np.float32))
    W2 = np.ascontiguousarray(np.asarray(W2, dtype=np.float32))
    W3 = np.ascontiguousarray(np.asarray(W3, dtype=np.float32))
    b1 = np.asarray(b1, dtype=np.float32)
    b2 = np.asarray(b2, dtype=np.float32)
    b3 = np.asarray(b3, dtype=np.float32)

    order = np.argsort(idx, kind="stable")
    counts = np.bincount(idx, minlength=P).tolist()
    # float32r matmuls reject odd moving dims (s3d3_mm_fp32r_restrictions):
    # pad each group to a multiple of 4 dead columns, skipped at scatter
    pcounts = [-(-n // 4) * 4 for n in counts]
    Bp = sum(pcounts)

    in_maps = _pack_inputs(
        latents, actions, order, counts, pcounts, Bp, W1, b1, W2, b2, W3, b3
    )
    nc = _build_program(pcounts, Bp, repeat=REPEAT)
    return nc, in_maps, order, counts, pcounts


def _scatter_out(results, order, counts, pcounts, B):
    out = np.empty((B, D), dtype=np.float32)
    keep = np.zeros(sum(pcounts), dtype=bool)
    po = 0
    for n, pn in zip(counts, pcounts):
        keep[po : po + n] = True
        po += pn
    for i in range(NCORES):
        yT = results[i]["yT"][:, keep]                # [Z, B] sorted order
        out[order, i * Z : (i + 1) * Z] = yT.T
    return out


def run_timed(nc, in_maps, iters=20):
    """Execute the finalized Bass program on the 8 cores via PJRT, timing
    repeated dispatches of the prebuilt executable (min over iters).
    Returns (per-core results, list of wall times in seconds)."""
    import time

    import jax
    from jax.experimental.shard_map import shard_map
    from jax.sharding import Mesh, NamedSharding, PartitionSpec

    from concourse import bass2jax, mybir as _mybir
    from concourse.bass2jax import _bass_exec_p, partition_id_tensor

    bass2jax.install_neuronx_cc_hook()
    n_cores = len(in_maps)

    partition_name = nc.partition_id_tensor.name if nc.partition_id_tensor else None
    in_names, out_names, out_avals, zero_outs = [], [], [], []
    for alloc in nc.m.functions[0].allocations:
        if not isinstance(alloc, _mybir.MemoryLocationSet):
            continue
        name = alloc.memorylocations[0].name
        if alloc.kind == "ExternalInput":
            if name != partition_name:
                in_names.append(name)
        elif alloc.kind == "ExternalOutput":
            out_names.append(name)
            shape = tuple(alloc.tensor_shape)
            dtype = _mybir.dt.np(alloc.dtype)
            out_avals.append(jax.core.ShapedArray(shape, dtype))
            zero_outs.append(np.zeros(shape, dtype))
    n_params = len(in_names)
    n_outs = len(out_avals)
    all_in_names = list(in_names) + out_names + (
        [partition_name] if partition_name else []
    )

    def _body(*args):
        operands = list(args)
        if partition_name is not None:
            operands.append(partition_id_tensor())
        outs = _bass_exec_p.bind(
            *operands,
            out_avals=tuple(out_avals),
            in_names=tuple(all_in_names),
            out_names=tuple(out_names),
            lowering_input_output_aliases=(),
            sim_require_finite=True,
            sim_require_nnan=True,
            nc=nc,
        )
        return tuple(outs)

    devices = jax.devices()[:n_cores]
    mesh = Mesh(np.asarray(devices), ("core",))
    spec = PartitionSpec("core")
    in_specs = (spec,) * (n_params + n_outs)
    out_specs = (spec,) * n_outs
    donate = tuple(range(n_params, n_params + n_outs))
    sharded = jax.jit(
        shard_map(_body, mesh=mesh, in_specs=in_specs, out_specs=out_specs,
                  check_rep=False),
        donate_argnums=donate,
        keep_unused=True,
    )
    sh = NamedSharding(mesh, spec)
    concat_in = [
        jax.device_put(
            np.concatenate([np.asarray(in_maps[c][nm]) for c in range(n_cores)], 0),
            sh,
        )
        for nm in in_names
    ]

    def fresh_zeros():
        return [
            jax.device_put(
                np.zeros((n_cores * z.shape[0], *z.shape[1:]), z.dtype), sh
            )
            for z in zero_outs
        ]

    out_arrs = sharded(*concat_in, *fresh_zeros())  # warmup + result
    jax.block_until_ready(out_arrs)
    results = [
        {
            nm: np.asarray(out_arrs[i]).reshape(n_cores, *out_avals[i].shape)[c]
            for i, nm in enumerate(out_names)
        }
        for c in range(n_cores)
    ]

    staged = [fresh_zeros() for _ in range(iters)]
    jax.block_until_ready(staged)
    import jax.numpy as jnp

    reduce_fn = jax.jit(lambda a: jnp.sum(a[:, :4]))
    times = []
    for z in staged:
        t0 = time.perf_counter()
        o = sharded(*concat_in, *z)
        float(reduce_fn(o[0]))  # tiny dependent reduction forces completion
        times.append(time.perf_counter() - t0)
    return results, times


def kernel(latents, actions, policy_indices, W1, b1, W2, b2, W3, b3):
    global LAST_RESULT
    nc, in_maps, order, counts, pcounts = _prepare(
        latents, actions, policy_indices, W1, b1, W2, b2, W3, b3
    )
    res = run_bass_kernel_spmd(nc, in_maps, list(range(NCORES)), trace=TRACE)
    LAST_RESULT = res
    return _scatter_out(
        res.results, order, counts, pcounts, np.asarray(latents).shape[0]
    )


# revision 33
# speedup vs baseline: 1.0026x; 1.0026x over previous
"""Inner-policy-sharded Trainium2 kernel for DecoupledDynamicsModel (MoE).

Model: B=8192 rows; each row selects one of P=8 outer policies via
policy_indices; the selected policy runs 8 inner MLPs (72 -> 512 -> 512 -> 64)
on (latent chunk, action) and the 8 inner outputs concatenate to 512 dims.

Sharding: by INNER policy. Core i computes inner MLP i for every row, using
the row's outer-policy weight set W*[outer, i]. Rows are sorted by outer
policy on the host so tokens form 8 contiguous groups; within a group the
weights are stationary. This gives perfect load balance (every core runs
exactly B tokens) and no capacity padding, unlike outer-expert sharding
which pads every core to max(group size).

On-chip layout is feature-major ([features(part), tokens(free)]) so no
transposes are needed between layers. Matmuls run in float32r (FP32 stored,
FP22 multiply, FP32 accumulate) at 1 cycle/row for moving dim >=256.
Bias+relu rides the PSUM->SBUF eviction, split across ACT (layer 1) and
DVE (layers 2 and 3) to balance engine load.

All 8 outer weight sets for this core's inner MLP are packed into eight
[128, 2825]-column SBUF tiles (W1|W2|W3|b1|b2|b3) loaded once up front;
x rides a separate [72, B] tile. Built on Bacc so multi-wait instructions
are legalized (TRN2 allows one sync wait per instruction).
"""

import sys

sys.path.insert(0, "/opt/trn_rl_repo")

import numpy as np

import concourse.bass as bass
from concourse import bacc
import concourse.mybir as mybir
import concourse.tile as tile
from concourse.bass import ts
from concourse.bass_utils import run_bass_kernel_spmd

P = 8          # outer policies == n_cores == inner MLPs per policy
Z = 64         # per-policy latent dim
D = P * Z      # 512
A = 8          # action dim
IN = Z + A     # 72, MLP input dim
H = 512        # hidden dim
NCORES = 8

# column layout of the [128, WBC] packed per-outer weight tile; ordered so
# a tiny leading DMA chunk (biases + W1 m-chunk 0) unblocks the first
# matmul+relu as early as possible
B1_OFF = 0          # [128, 4] (m-chunk per column)
B2_OFF = 4          # [128, 4]
B3_OFF = 8          # [64, 1] (rows 64.. zero)
W1_OFF = 16         # [72, 512] (rows 72.. zero), cols 16..528
W2_OFF = 528        # 4 k-chunks of [128, 512]
W3_OFF = 2576       # 4 k-chunks of [128, 64]
WBC = 2832

F32 = mybir.dt.float32
F32R = mybir.dt.float32r
RELU = mybir.ActivationFunctionType.Relu

TRACE = False
REPEAT = 1
LAST_RESULT = None


def _group_tiles(counts):
    """Token tiles for the sorted stream: each tile stays inside one outer-
    policy group; tiles <=512 and >=256 where possible (float32r needs a
    moving dim >=256 for full rate)."""
    tiles = []
    off = 0
    for g, n in enumerate(counts):
        r = n
        if g == 0 and r > 768:
            # small leading tile: compute starts on fewer loaded bytes and
            # the first relu chain is shorter, so the pipeline ramps sooner
            tiles.append((g, off, 256))
            off += 256
            r -= 256
        while r > 0:
            if r > 768:
                t = 512
            elif r > 512:
                t = r - 256
            else:
                t = r
            tiles.append((g, off, t))
            off += t
            r -= t
    return tiles


def _build_program(counts, B, repeat=1):
    tiles = _group_tiles(counts)
    nc = bacc.Bacc()

    xTd = nc.declare_dram_parameter("xT", [IN, B], F32R, isOutput=False)
    wbd = nc.declare_dram_parameter("wb", [P, 128, WBC], F32R, isOutput=False)
    yTd = nc.declare_dram_parameter("yT", [Z, B], F32, isOutput=True)

    with tile.TileContext(nc) as tc:
        with (
            tc.tile_pool(name="weights", bufs=8) as wpool,
            tc.tile_pool(name="xs", bufs=1) as xpool,
            tc.tile_pool(name="hs", bufs=4) as hpool,
            tc.tile_pool(name="ys", bufs=8) as ypool,
            tc.tile_pool(name="ps1", bufs=4, space="PSUM") as pspool1,
            tc.tile_pool(name="ps2", bufs=3, space="PSUM") as pspool2,
            tc.tile_pool(name="ps3", bufs=1, space="PSUM") as pspool3,
        ):
            for _rep in range(repeat):
                xt = xpool.tile([IN, B], F32R, tag="x")
                wbs = []
                for _g in range(P):
                    wb_t = wpool.tile([128, WBC], F32R, tag="wb")
                    wbs.append(wb_t)
                # DMAs effectively drain through one serial pipe, so emit in
                # NEED order: tile 0's x columns and W1 first, W2 k-chunks
                # interleaved with the next x spans, bulk weights last.
                wb0_cuts = [0, W1_OFF + 256, W2_OFF, W2_OFF + 512,
                            W2_OFF + 1024, W2_OFF + 1536, W3_OFF, WBC]
                x_cuts = [0, 256, 1024, 2048, 4096, B]
                x_cuts = sorted(set(min(c, B) for c in x_cuts))
                xsp = [c for c in zip(x_cuts[:-1], x_cuts[1:]) if c[1] > c[0]]
                w0 = list(zip(wb0_cuts[:-1], wb0_cuts[1:]))
                # serial DMA pipe: emit in need order. Group 0's chunks and
                # the first x spans lead; each later group's chunks follow in
                # group order (just-in-time), remaining x spans interleaved.
                # the first x span rides SWDGE (gpsimd is otherwise idle):
                # it transfers in parallel with the HWDGE weight pipe, so the
                # first matmul is gated only by the leading weight chunk
                nc.gpsimd.dma_start(
                    xt[:, xsp[0][0] : xsp[0][1]], xTd[:, xsp[0][0] : xsp[0][1]]
                )
                # second W1 chunk also on SWDGE: needed ~4us in, so its
                # higher first-byte latency is hidden, and it leaves the
                # HWDGE pipe free for the W2 chunks
                nc.gpsimd.dma_start(wbs[0][:, w0[1][0] : w0[1][1]],
                                    wbd[0, :, w0[1][0] : w0[1][1]])
                order = [(0, w0[0]), (0, w0[2])]
                if len(xsp) > 1:
                    order.append(("x", xsp[1]))
                order += [(0, c) for c in w0[3:]]
                gchunks = [(0, 528), (528, 1040), (1040, 1552), (1552, 2064),
                           (2064, 2576), (2576, WBC)]
                for g in range(1, P):
                    order += [(g, c) for c in gchunks]
                    xi = 1 + g
                    if xi < len(xsp):
                        order.append(("x", xsp[xi]))
                for kind, (c0, c1) in order:
                    if c1 <= c0:
                        continue
                    if kind == "x":
                        nc.sync.dma_start(xt[:, c0:c1], xTd[:, c0:c1])
                    else:
                        nc.sync.dma_start(wbs[kind][:, c0:c1], wbd[kind, :, c0:c1])

                for (g, t0, tw) in tiles:
                    wb = wbs[g]
                    h1 = hpool.tile([128, 4, 512], F32R, tag="h1")
                    h2 = hpool.tile([128, 4, 512], F32R, tag="h2")
                    # layer 1: h1 = relu(W1.T @ x + b1), K=72
                    for m in range(4):
                        ps = pspool1.tile([128, 512], F32, tag="ps1")
                        nc.tensor.matmul(
                            ps[:, :tw],
                            wb[0:IN, W1_OFF + m * 128 : W1_OFF + (m + 1) * 128],
                            xt[:, t0 : t0 + tw],
                            start=True,
                            stop=True,
                        )
                        nc.scalar.activation(
                            h1[:, m, :tw], ps[:, :tw], RELU,
                            bias=wb[:, B1_OFF + m : B1_OFF + m + 1].bitcast(F32),
                        )
                    # layer 2: h2 = relu(W2.T @ h1 + b2), K=512 over 4 chunks
                    for m in range(4):
                        ps = pspool2.tile([128, 512], F32, tag="ps2")
                        for k in range(4):
                            nc.tensor.matmul(
                                ps[:, :tw],
                                wb[:, W2_OFF + k * 512 + m * 128 : W2_OFF + k * 512 + (m + 1) * 128],
                                h1[:, k, :tw],
                                start=(k == 0),
                                stop=(k == 3),
                            )
                        nc.vector.tensor_scalar(
                            h2[:, m, :tw],
                            ps[:, :tw],
                            wb[:, B2_OFF + m : B2_OFF + m + 1].bitcast(F32),
                            0.0,
                            mybir.AluOpType.add,
                            mybir.AluOpType.max,
                        )
                    # layer 3: y = W3.T @ h2 + b3, M=64
                    ps = pspool3.tile([Z, 512], F32, tag="ps3")
                    for k in range(4):
                        nc.tensor.matmul(
                            ps[:, :tw],
                            wb[:, W3_OFF + k * Z : W3_OFF + (k + 1) * Z],
                            h2[:, k, :tw],
                            start=(k == 0),
                            stop=(k == 3),
                        )
                    y = ypool.tile([Z, 512], F32, tag="y")
                    nc.vector.tensor_scalar(
                        y[:, :tw], ps[:, :tw],
                        wb[0:Z, B3_OFF : B3_OFF + 1].bitcast(F32),
                        None,
                        mybir.AluOpType.add,
                    )
                    nc.sync.dma_start(yTd[:, t0 : t0 + tw], y[:, :tw])

    nc.finalize()
    return nc


def _pack_inputs(latents, actions, order, counts, pcounts, Bp,
                 W1, b1, W2, b2, W3, b3):
    """Per-core inputs. Core i: xT = [latent chunk i; action] for all rows in
    sorted order (groups padded to pcounts); wb[g] = weights of (outer g,
    inner i)."""
    lat_s = latents[order]                       # [B, 512]
    act_s = actions[order]                       # [B, 8]
    spans = []                                   # (padded off, raw off, n)
    po = ro = 0
    for n, pn in zip(counts, pcounts):
        spans.append((po, ro, n))
        po += pn
        ro += n
    in_maps = []
    for i in range(NCORES):
        xT = np.zeros((IN, Bp), dtype=np.float32)
        for po, ro, n in spans:
            xT[:Z, po : po + n] = lat_s[ro : ro + n, i * Z : (i + 1) * Z].T
            xT[Z:, po : po + n] = act_s[ro : ro + n].T

        wb = np.zeros((P, 128, WBC), dtype=np.float32)
        wb[:, :IN, W1_OFF : W1_OFF + 512] = W1[:, i]           # [P, 72, 512]
        wb[:, :, W2_OFF : W2_OFF + 2048] = (
            W2[:, i].reshape(P, 4, 128, H).transpose(0, 2, 1, 3).reshape(P, 128, 2048)
        )
        wb[:, :, W3_OFF : W3_OFF + 256] = (
            W3[:, i].reshape(P, 4, 128, Z).transpose(0, 2, 1, 3).reshape(P, 128, 256)
        )
        wb[:, :, B1_OFF : B1_OFF + 4] = b1[:, i].reshape(P, 4, 128).transpose(0, 2, 1)
        wb[:, :, B2_OFF : B2_OFF + 4] = b2[:, i].reshape(P, 4, 128).transpose(0, 2, 1)
        wb[:, :Z, B3_OFF] = b3[:, i]

        in_maps.append({"xT": xT, "wb": wb})
    return in_maps


def _prepare(latents, actions, policy_indices, W1, b1, W2, b2, W3, b3):
    latents = np.asarray(latents, dtype=np.float32)
    actions = np.asarray(actions, dtype=np.float32)
    idx = np.asarray(policy_indices).astype(np.int64)
    W1 = np.ascontiguousarray(np.asarray(W1, dtype=np.float32))
    W2 = np.ascontiguousarray(np.asarray(W2, dtype=np.float32))
    W3 = np.ascontiguousarray(np.asarray(W3, dtype=np.float32))
    b1 = np.asarray(b1, dtype=np.float32)
    b2 = np.asarray(b2, dtype=np.float32)
    b3 = np.asarray(b3, dtype=np.float32)

    order = np.argsort(idx, kind="stable")
    counts = np.bincount(idx, minlength=P).tolist()
    # float32r matmuls reject odd moving dims (s3d3_mm_fp32r_restrictions):
    # pad each group to a multiple of 4 dead columns, skipped at scatter
    pcounts = [-(-n // 4) * 4 for n in counts]
    Bp = sum(pcounts)

    in_maps = _pack_inputs(
        latents, actions, order, counts, pcounts, Bp, W1, b1, W2, b2, W3, b3
    )
    nc = _build_program(pcounts, Bp, repeat=REPEAT)
    return nc, in_maps, order, counts, pcounts


def _scatter_out(results, order, counts, pcounts, B):
    out = np.empty((B, D), dtype=np.float32)
    keep = np.zeros(sum(pcounts), dtype=bool)
    po = 0
    for n, pn in zip(counts, pcounts):
        keep[po : po + n] = True
        po += pn
    for i in range(NCORES):
        yT = results[i]["yT"][:, keep]                # [Z, B] sorted order
        out[order, i * Z : (i + 1) * Z] = yT.T
    return out


def run_timed(nc, in_maps, iters=20):
    """Execute the finalized Bass program on the 8 cores via PJRT, timing
    repeated dispatches of the prebuilt executable (min over iters).
    Returns (per-core results, list of wall times in seconds)."""
    import time

    import jax
    from jax.experimental.shard_map import shard_map
    from jax.sharding import Mesh, NamedSharding, PartitionSpec

    from concourse import bass2jax, mybir as _mybir
    from concourse.bass2jax import _bass_exec_p, partition_id_tensor

    bass2jax.install_neuronx_cc_hook()
    n_cores = len(in_maps)

    partition_name = nc.partition_id_tensor.name if nc.partition_id_tensor else None
    in_names, out_names, out_avals, zero_outs = [], [], [], []
    for alloc in nc.m.functions[0].allocations:
        if not isinstance(alloc, _mybir.MemoryLocationSet):
            continue
        name = alloc.memorylocations[0].name
        if alloc.kind == "ExternalInput":
            if name != partition_name:
                in_names.append(name)
        elif alloc.kind == "ExternalOutput":
            out_names.append(name)
            shape = tuple(alloc.tensor_shape)
            dtype = _mybir.dt.np(alloc.dtype)
            out_avals.append(jax.core.ShapedArray(shape, dtype))
            zero_outs.append(np.zeros(shape, dtype))
    n_params = len(in_names)
    n_outs = len(out_avals)
    all_in_names = list(in_names) + out_names + (
        [partition_name] if partition_name else []
    )

    def _body(*args):
        operands = list(args)
        if partition_name is not None:
            operands.append(partition_id_tensor())
        outs = _bass_exec_p.bind(
            *operands,
            out_avals=tuple(out_avals),
            in_names=tuple(all_in_names),
            out_names=tuple(out_names),
            lowering_input_output_aliases=(),
            sim_require_finite=True,
            sim_require_nnan=True,
            nc=nc,
        )
        return tuple(outs)

    devices = jax.devices()[:n_cores]
    mesh = Mesh(np.asarray(devices), ("core",))
    spec = PartitionSpec("core")
    in_specs = (spec,) * (n_params + n_outs)
    out_specs = (spec,) * n_outs
    donate = tuple(range(n_params, n_params + n_outs))
    sharded = jax.jit(
        shard_map(_body, mesh=mesh, in_specs=in_specs, out_specs=out_specs,
                  check_rep=False),
        donate_argnums=donate,
        keep_unused=True,
    )
    sh = NamedSharding(mesh, spec)
    concat_in = [
        jax.device_put(
            np.concatenate([np.asarray(in_maps[c][nm]) for c in range(n_cores)], 0),
            sh,
        )
        for nm in in_names
    ]

    def fresh_zeros():
        return [
            jax.device_put(
                np.zeros((n_cores * z.shape[0], *z.shape[1:]), z.dtype), sh
            )
            for z in zero_outs
        ]

    out_arrs = sharded(*concat_in, *fresh_zeros())  # warmup + result
    jax.block_until_ready(out_arrs)
    results = [
        {
            nm: np.asarray(out_arrs[i]).reshape(n_cores, *out_avals[i].shape)[c]
            for i, nm in enumerate(out_names)
        }
        for c in range(n_cores)
    ]

    staged = [fresh_zeros() for _ in range(iters)]
    jax.block_until_ready(staged)
    import jax.numpy as jnp

    reduce_fn = jax.jit(lambda a: jnp.sum(a[:, :4]))
    times = []
    for z in staged:
        t0 = time.perf_counter()
        o = sharded(*concat_in, *z)
        float(reduce_fn(o[0]))  # tiny dependent reduction forces completion
        times.append(time.perf_counter() - t0)
    return results, times


def kernel(latents, actions, policy_indices, W1, b1, W2, b2, W3, b3):
    global LAST_RESULT
    nc, in_maps, order, counts, pcounts = _prepare(
        latents, actions, policy_indices, W1, b1, W2, b2, W3, b3
    )
    res = run_bass_kernel_spmd(nc, in_maps, list(range(NCORES)), trace=TRACE)
    LAST_RESULT = res
    return _scatter_out(
        res.results, order, counts, pcounts, np.asarray(latents).shape[0]
    )
